# revision 31
# baseline (speedup 1.0000x reference)
"""LlamaAttention (GQA, no mask) on 8 Trainium2 NeuronCores.

Sharding: 8 cores = 2 (batch) x 4 (head groups of 8 q heads / 2 kv heads).

Precision plan:
  Projections (q/k/v/o) run as fp8-e4m3 DoubleRow matmuls with 3-term
  operand splits: a = ah + al, w = wh + wl (each digit e4m3), computing
  ah*wh + ah*wl + al*wh. With weights pre-scaled x32 (power of 2, undone
  in the psum->sbuf copy) the split is MORE accurate than bf16 while
  DoubleRow (256-deep contraction at 0.5 cyc/row) runs 2.67x faster than
  bf16 for the same contraction. x and all weights are split host-side;
  the o-proj operand (attn out) is split on-device by two DVE ops.
  Scores / exp / attn@v stay bf16: fp8 on the softmax paths measured
  2.4-3.8e-2 rms error (over the 2e-2 budget), and 3-term splits cost
  more than bf16 there (K=64 scores already half-utilize the PE).

Per core:
  q/k/v projections fp8-DR fused per 512-wide q block.
  scores   s^T[k,q] = matmul(kdup [64,128] chunk, q_sb [64,512])  bf16
  exp      ACT engine: exp(s/8 - 2) -> bf16 (shift cancels in softmax)
  attn@v   REVERSED: p chunk is the (free) stationary operand, v the moving
           one -> out [q, 65] costs 65 rows/chunk instead of 512. Column 64
           accumulates the softmax denominator (ones column), which lands
           per-PARTITION so normalize is a cheap per-partition tensor_scalar
           (x4, folded into the same op, to center the fp8 split).
  at^T     PE transpose (identity matmul) back to [attn, s]; split into
           e4m3 hi/lo digits by DVE.
  o_proj   fp8-DR, interleaved per 512-wide q block; output f32 (x 1/128).
Host sums the 4 head-group partials per batch.
"""

import numpy as np
import ml_dtypes

S = 2048          # sequence length
D = 2048          # model dim
HD = 64           # head dim
QC = 512          # q cols per core (8 heads)
KVC = 128         # kv cols per core (2 kv heads)
DC = 16           # 128-contraction chunks
SC = 16           # 128-seq chunks
NJQ = 4           # q blocks
QB = 512          # q block width
SCALE = HD ** -0.5
WS = 32.0         # weight pre-scale (power of 2)
AS = 4.0          # attn-out pre-scale for the o-proj fp8 split

# deferred-load release times (ms into the schedule), tuned from traces
T_WQ = 0.002      # wq columns 128:512 (needed by q-block 2)
T_WV = 0.0025     # wv (first vproj units)
T_EYE = 0.006     # transpose identity (first fini)
T_WO = 0.009      # wo (first oproj units)

_CACHE = {}


def _build():
    import concourse.mybir as mybir
    import concourse.tile as tile
    from concourse import bacc

    f32 = mybir.dt.float32
    bf16 = mybir.dt.bfloat16
    f8 = mybir.dt.float8e4
    Exp = mybir.ActivationFunctionType.Exp
    DR = mybir.MatmulPerfMode.DoubleRow

    nc = bacc.Bacc("TRN2", target_bir_lowering=False, debug=False, num_devices=8)

    xh = nc.dram_tensor("xh", [D, S], f8, kind="ExternalInput").ap()
    xl = nc.dram_tensor("xl", [D, S], f8, kind="ExternalInput").ap()
    wqh = nc.dram_tensor("wqh", [D, QC], f8, kind="ExternalInput").ap()
    wql = nc.dram_tensor("wql", [D, QC], f8, kind="ExternalInput").ap()
    wkh = nc.dram_tensor("wkh", [D, KVC], f8, kind="ExternalInput").ap()
    wkl = nc.dram_tensor("wkl", [D, KVC], f8, kind="ExternalInput").ap()
    wvh = nc.dram_tensor("wvh", [D, KVC], f8, kind="ExternalInput").ap()
    wvl = nc.dram_tensor("wvl", [D, KVC], f8, kind="ExternalInput").ap()
    woh = nc.dram_tensor("woh", [QC, D], f8, kind="ExternalInput").ap()
    wol = nc.dram_tensor("wol", [QC, D], f8, kind="ExternalInput").ap()
    eye = nc.dram_tensor("eye", [128, 128], bf16, kind="ExternalInput").ap()
    out = nc.dram_tensor("out", [S, D], f32, kind="ExternalOutput").ap()

    MUL = mybir.AluOpType.mult
    SUB = mybir.AluOpType.subtract

    with tile.TileContext(nc) as tc:
        with tc.tile_pool(name="const", bufs=1) as const, \
             tc.tile_pool(name="mm", bufs=2, space="PSUM") as mm, \
             tc.tile_pool(name="qpp", bufs=1, space="PSUM") as qpp, \
             tc.tile_pool(name="av", bufs=2, space="PSUM") as av, \
             tc.tile_pool(name="tr", bufs=1, space="PSUM") as trp, \
             tc.tile_pool(name="qs", bufs=3) as qs, \
             tc.tile_pool(name="pp", bufs=6) as pp, \
             tc.tile_pool(name="dn", bufs=8) as dn, \
             tc.tile_pool(name="ar", bufs=2) as ar, \
             tc.tile_pool(name="ev", bufs=6) as ev:

            # ---------------- resident inputs ----------------
            wkh_all = const.tile([128, DC, KVC], f8, tag="wkh_all")
            nc.sync.dma_start(out=wkh_all[:], in_=wkh.rearrange("(c p) n -> p c n", p=128))
            wkl_all = const.tile([128, DC, KVC], f8, tag="wkl_all")
            nc.sync.dma_start(out=wkl_all[:], in_=wkl.rearrange("(c p) n -> p c n", p=128))
            wqh_all = const.tile([128, DC, QC], f8, tag="wqh_all")
            wql_all = const.tile([128, DC, QC], f8, tag="wql_all")
            wqh_re = wqh.rearrange("(c p) n -> p c n", p=128)
            wql_re = wql.rearrange("(c p) n -> p c n", p=128)
            nc.sync.dma_start(out=wqh_all[:, :, 0:128], in_=wqh_re[:, :, 0:128])
            nc.sync.dma_start(out=wql_all[:, :, 0:128], in_=wql_re[:, :, 0:128])
            xh_all = const.tile([128, DC, S], f8, tag="xh_all")
            xl_all = const.tile([128, DC, S], f8, tag="xl_all")
            xh_re = xh.rearrange("(c p) s -> p c s", p=128)
            xl_re = xl.rearrange("(c p) s -> p c s", p=128)
            # x streams on the (otherwise idle) gpsimd DMA queue in seq-column
            # blocks (all DC rows per block): k-proj s4 / q-proj jq blocks
            # depend only on their own columns, so compute streams behind the
            # DMA with ~0.5MB granularity instead of waiting for all of x.
            for c in range(4):
                sl = slice(c * 512, (c + 1) * 512)
                nc.gpsimd.dma_start(out=xh_all[:, :, sl], in_=xh_re[:, :, sl])
                nc.gpsimd.dma_start(out=xl_all[:, :, sl], in_=xl_re[:, :, sl])
            wvh_all = const.tile([128, DC, KVC], f8, tag="wvh_all")
            wvl_all = const.tile([128, DC, KVC], f8, tag="wvl_all")
            eye_t = const.tile([128, 128], bf16, tag="eye_t")
            woh_all = const.tile([128, QC // 128, D], f8, tag="woh_all")
            wol_all = const.tile([128, QC // 128, D], f8, tag="wol_all")

            # persistent intermediates
            kdup = const.tile([128, 2, S], bf16, tag="kdup")  # kv head x both halves
            k_sb = const.tile([128, S], bf16, tag="k_sb")
            vv = const.tile([128, SC, 2, 80], bf16, tag="vv")  # [k, sc, kv, 64v+ones]
            at8h = const.tile([128, 4, S], f8, tag="at8h")    # attn out^T hi digit
            at8l = const.tile([128, 4, S], f8, tag="at8l")    # attn out^T lo digit
            bias_m2 = const.tile([128, 1], f32, tag="bias_m2")
            nc.vector.memset(bias_m2[:], -2.0)
            warm = const.tile([128, 1], bf16, tag="warm")
            nc.scalar.activation(warm[:], bias_m2[:], Exp, bias=bias_m2[:],
                                 scale=SCALE)
            nc.vector.memset(vv[:, :, :, 64:65], 1.0)

            # ---------------- k projection: k^T [128 kcols, S] ----------------
            for s4 in range(4):
                sl = slice(s4 * 512, (s4 + 1) * 512)
                kp = mm.tile([128, 2, 512], f32, tag="mm4k")
                first = True
                for wk_t, x_t in ((wkh_all, xh_all), (wkl_all, xh_all),
                                  (wkh_all, xl_all)):
                    for d in range(DC // 2):
                        nc.tensor.matmul(kp[:, 0, :],
                                         wk_t[:, 2 * d:2 * d + 2, :],
                                         x_t[:, 2 * d:2 * d + 2, sl],
                                         start=first,
                                         stop=(wk_t is wkh_all
                                               and x_t is xl_all
                                               and d == DC // 2 - 1),
                                         perf_mode=DR)
                        first = False
                nc.vector.tensor_scalar(k_sb[:, sl], kp[:, 0, :],
                                        1.0 / WS, None, MUL)
                # duplicate each kv head into both partition halves (per s4,
                # so block-0 scores can start before the later s4 finish)
                for kv in range(2):
                    for half in range(2):
                        nc.sync.dma_start(
                            out=kdup[half * 64:half * 64 + 64, kv, sl],
                            in_=k_sb[kv * 64:kv * 64 + 64, sl])

            # ------------- q-proj pipeline stage -------------
            blocks = [(jq, qm) for jq in range(NJQ) for qm in range(4)]
            q_sb_of = {}

            def qstage(i):
                jq, qm = blocks[i]
                qsl = slice(jq * QB, (jq + 1) * QB)
                qp = qpp.tile([128, 512], f32, tag="qp")
                first = True
                for wq_t, x_t in ((wqh_all, xh_all), (wql_all, xh_all),
                                  (wqh_all, xl_all)):
                    for d in range(DC // 2):
                        nc.tensor.matmul(qp[:],
                                         wq_t[:, 2 * d:2 * d + 2,
                                              qm * 128:(qm + 1) * 128],
                                         x_t[:, 2 * d:2 * d + 2, qsl],
                                         start=first,
                                         stop=(wq_t is wqh_all
                                               and x_t is xl_all
                                               and d == DC // 2 - 1),
                                         perf_mode=DR)
                        first = False
                q_sb = qs.tile([128, QB], bf16, tag="q_sb")
                nc.vector.tensor_scalar(q_sb[:], qp[:], 1.0 / WS, None, MUL)
                q_sb_of[i] = q_sb

            qstage(0)
            # deferred loads ride the ACT DMA queue (idle until exps start,
            # and input loads have no waits so they can't head-of-line block
            # it); the SP queue stays clear for the k_sb-gated kdup copies.
            with tc.tile_wait_until(T_WQ):
                nc.sync.dma_start(out=wqh_all[:, :, 128:QC],
                                    in_=wqh_re[:, :, 128:QC])
                nc.sync.dma_start(out=wql_all[:, :, 128:QC],
                                    in_=wql_re[:, :, 128:QC])
            with tc.tile_wait_until(T_WV):
                nc.sync.dma_start(out=wvh_all[:],
                                    in_=wvh.rearrange("(c p) n -> p c n", p=128))
                nc.sync.dma_start(out=wvl_all[:],
                                    in_=wvl.rearrange("(c p) n -> p c n", p=128))
            with tc.tile_wait_until(T_EYE):
                nc.sync.dma_start(out=eye_t[:], in_=eye)
            with tc.tile_wait_until(T_WO):
                nc.sync.dma_start(out=woh_all[:],
                                    in_=woh.rearrange("(c p) n -> p c n", p=128))
                nc.sync.dma_start(out=wol_all[:],
                                    in_=wol.rearrange("(c p) n -> p c n", p=128))

            # ---------------- v projection (deferred fillers) ----------------
            def vproj_unit(sc):
                vp2 = mm.tile([128, 2, 512], f32, tag="mm4k", name="vp")
                vp = vp2[:, 0, :]
                first = True
                for x_t, wv_t in ((xh_all, wvh_all), (xh_all, wvl_all),
                                  (xl_all, wvh_all)):
                    for d in range(DC // 2):
                        nc.tensor.matmul(vp[:, 0:128],
                                         x_t[:, 2 * d:2 * d + 2,
                                             sc * 128:(sc + 1) * 128],
                                         wv_t[:, 2 * d:2 * d + 2, :],
                                         start=first,
                                         stop=(x_t is xl_all
                                               and d == DC // 2 - 1),
                                         perf_mode=DR)
                        first = False
                nc.vector.tensor_scalar(vv[:, sc, 0, 0:64], vp[:, 0:64],
                                        1.0 / WS, None, MUL)
                nc.vector.tensor_scalar(vv[:, sc, 1, 0:64], vp[:, 64:128],
                                        1.0 / WS, None, MUL)

            vfill = list(range(SC))

            # ------------- fused attention + o_proj -------------
            def oproj_unit(sm, q4, tail=0):
                op2 = mm.tile([128, 2, 512], f32, tag="mm4k", name="opt")
                op = op2[:, 0, :]
                o = q4 * 512
                first = True
                for a_t, wo_t in ((at8h, woh_all), (at8h, wol_all),
                                  (at8l, woh_all)):
                    for c in range(2):
                        nc.tensor.matmul(op[:],
                                         a_t[:, 2 * c:2 * c + 2,
                                             sm * 128:(sm + 1) * 128],
                                         wo_t[:, 2 * c:2 * c + 2, o:o + 512],
                                         start=first,
                                         stop=(a_t is at8l and c == 1),
                                         perf_mode=DR)
                        first = False
                o_sb = ev.tile([128, 512], f32, tag="o_sb")
                nc.vector.tensor_scalar(o_sb[:], op[:], 1.0 / (WS * AS),
                                        None, MUL)
                nc.sync.dma_start(
                    out=out[sm * 128:(sm + 1) * 128, q4 * 512:(q4 + 1) * 512],
                    in_=o_sb[:])

            # Flat task stream over (block, head, kc-pair); attn@v trails by
            # ATTNV_LAG tasks so its exp-wait never blocks scores in the PE
            # FIFO; o_proj/v_proj units fill PE slack mid-head.
            ATTNV_LAG = 4
            pending = []
            attnv_q = []
            fini_q = []

            def drain_attnv():
                o_ps, p_ap, pr, kv, fini = attnv_q.pop(0)
                for j2 in range(2):
                    kc = 2 * pr + j2
                    for qc in range(4):
                        nc.tensor.matmul(
                            o_ps[:, qc, 0:65],
                            p_ap[:, j2, qc * 128:(qc + 1) * 128],
                            vv[:, kc, kv, 0:65],
                            start=(kc == 0 and qc == 0),
                            stop=(kc == SC - 1 and qc == 3),
                            skip_group_check=True)
                if fini is not None:
                    fini_q.append(fini)

            def drain_fini():
                o_ps, hb, cc, jq = fini_q.pop(0)
                atr = ar.tile([128, 4, HD], bf16, tag="atr")
                for qc in range(4):
                    rc = dn.tile([128, 1], f32, tag="rc")
                    nc.vector.reciprocal(rc[:], o_ps[:, qc, 64:65])
                    # atr = (o_ps / denom) * AS  (AS centers the fp8 split)
                    nc.vector.tensor_scalar(atr[:, qc, :], o_ps[:, qc, 0:64],
                                            rc[:], AS, MUL, MUL)
                tr = trp.tile([128, 4, 128], bf16, tag="tr")
                for qc in range(4):
                    nc.tensor.matmul(tr[hb:hb + 64, qc, :], atr[:, qc, :],
                                     eye_t[:], is_transpose=True,
                                     tile_position=(0, hb))
                sl = slice(jq * QB, (jq + 1) * QB)
                nc.vector.tensor_copy(at8h[hb:hb + 64, cc, sl],
                                      tr[hb:hb + 64, :, :])
                nc.vector.tensor_tensor(at8l[hb:hb + 64, cc, sl],
                                        tr[hb:hb + 64, :, :],
                                        at8h[hb:hb + 64, cc, sl], SUB)

            for i, (jq, qm) in enumerate(blocks):
                if i + 1 < len(blocks):
                    qstage(i + 1)
                q_sb = q_sb_of.pop(i)
                for h2 in range(2):
                    l = 2 * qm + h2
                    kv = l // 4
                    hb = 64 * (l % 2)
                    cc = l // 2
                    qb = 64 * h2
                    o_ps = av.tile([128, 4, 80], f32, tag="o_ps")
                    for pr in range(8):
                        scp = mm.tile([128, 2, 512], f32, tag="mm4k")
                        for j2 in range(2):
                            kc = 2 * pr + j2
                            nc.tensor.matmul(
                                scp[:, j2, :],
                                kdup[qb:qb + 64, kv, kc * 128:(kc + 1) * 128],
                                q_sb[qb:qb + 64, :],
                                start=True, stop=True)
                        p4 = pp.tile([128, 2, QB], bf16, tag="p4")
                        nc.scalar.activation(p4[:], scp[:], Exp,
                                             bias=bias_m2[:], scale=SCALE)
                        fini = (o_ps, hb, cc, jq) if pr == 7 else None
                        attnv_q.append((o_ps, p4, pr, kv, fini))
                        if len(attnv_q) > ATTNV_LAG:
                            drain_attnv()
                        while fini_q:
                            drain_fini()
                        for _ in range(2):
                            if vfill:
                                vproj_unit(vfill.pop(0))
                        if pr in (3, 6) and pending:
                            oproj_unit(*pending.pop(0))
                if qm == 3:
                    pending.extend(((jq * 4 + smq, q4)
                                    for smq in range(4) for q4 in range(4)))
            while attnv_q:
                drain_attnv()
            while fini_q:
                drain_fini()
            for sm, q4 in pending:
                oproj_unit(sm, q4)

    nc.compile()
    return nc


def _get_nc():
    if "nc" not in _CACHE:
        _CACHE["nc"] = _build()
    return _CACHE["nc"]


def _split8(a):
    e4m3 = ml_dtypes.float8_e4m3
    hi = a.astype(e4m3)
    lo = (a - hi.astype(np.float32)).astype(e4m3)
    return hi, lo


def kernel(x, wq, wk, wv, wo):
    from concourse.bass_utils import run_bass_kernel_spmd

    bf16 = ml_dtypes.bfloat16
    nc = _get_nc()

    xnp, wqnp, wknp, wvnp, wonp = (
        np.asarray(a, dtype=np.float32) for a in (x, wq, wk, wv, wo))
    eye = np.eye(128, dtype=bf16)

    xs = [_split8(np.ascontiguousarray(xnp[b].T)) for b in range(2)]
    gsplits = []
    for g in range(4):
        gsplits.append({
            "wq": _split8(wqnp[:, g * QC:(g + 1) * QC] * WS),
            "wk": _split8(wknp[:, g * KVC:(g + 1) * KVC] * WS),
            "wv": _split8(wvnp[:, g * KVC:(g + 1) * KVC] * WS),
            "wo": _split8(wonp[g * QC:(g + 1) * QC, :] * WS),
        })

    in_maps = []
    for core in range(8):
        b, g = core // 4, core % 4
        gs = gsplits[g]
        in_maps.append({
            "xh": xs[b][0], "xl": xs[b][1],
            "wqh": gs["wq"][0], "wql": gs["wq"][1],
            "wkh": gs["wk"][0], "wkl": gs["wk"][1],
            "wvh": gs["wv"][0], "wvl": gs["wv"][1],
            "woh": gs["wo"][0], "wol": gs["wo"][1],
            "eye": eye,
        })

    res = run_bass_kernel_spmd(nc, in_maps, core_ids=list(range(8)))
    outs = [res.results[c]["out"] for c in range(8)]
    full = np.empty((2, S, D), np.float32)
    full[0] = outs[0] + outs[1] + outs[2] + outs[3]
    full[1] = outs[4] + outs[5] + outs[6] + outs[7]
    return full


# revision 34
# speedup vs baseline: 1.1427x; 1.1427x over previous
"""LlamaAttention (GQA, no mask) on 8 Trainium2 NeuronCores.

Sharding: 8 cores = 2 (batch) x 4 (head groups of 8 q heads / 2 kv heads).

Precision plan:
  Projections (q/k/v/o) run as fp8-e4m3 DoubleRow matmuls with 3-term
  operand splits: a = ah + al, w = wh + wl (each digit e4m3), computing
  ah*wh + ah*wl + al*wh. With weights pre-scaled x32 (power of 2, undone
  in the psum->sbuf copy) the split is MORE accurate than bf16 while
  DoubleRow (256-deep contraction at 0.5 cyc/row) runs 2.67x faster than
  bf16 for the same contraction. x and all weights are split host-side;
  the o-proj operand (attn out) is split on-device by two DVE ops.
  Scores / exp / attn@v stay bf16: fp8 on the softmax paths measured
  2.4-3.8e-2 rms error (over the 2e-2 budget), and 3-term splits cost
  more than bf16 there (K=64 scores already half-utilize the PE).

Per core:
  q/k/v projections fp8-DR fused per 512-wide q block.
  scores   s^T[k,q] = matmul(kdup [64,128] chunk, q_sb [64,512])  bf16
  exp      ACT engine: exp(s/8 - 2) -> bf16 (shift cancels in softmax)
  attn@v   REVERSED: p chunk is the (free) stationary operand, v the moving
           one -> out [q, 65] costs 65 rows/chunk instead of 512. Column 64
           accumulates the softmax denominator (ones column), which lands
           per-PARTITION so normalize is a cheap per-partition tensor_scalar
           (x4, folded into the same op, to center the fp8 split).
  at^T     PE transpose (identity matmul) back to [attn, s]; split into
           e4m3 hi/lo digits by DVE.
  o_proj   fp8-DR, interleaved per 512-wide q block; output f32 (x 1/128).
Host sums the 4 head-group partials per batch.
"""

import numpy as np
import ml_dtypes

S = 2048          # sequence length
D = 2048          # model dim
HD = 64           # head dim
QC = 512          # q cols per core (8 heads)
KVC = 128         # kv cols per core (2 kv heads)
DC = 16           # 128-contraction chunks
SC = 16           # 128-seq chunks
NJQ = 4           # q blocks
QB = 512          # q block width
SCALE = HD ** -0.5
WS = 32.0         # weight pre-scale (power of 2)
AS = 4.0          # attn-out pre-scale for the o-proj fp8 split

# deferred-load release times (ms into the schedule), tuned from traces
T_WQ = 0.006      # wq columns 128:512 (needed by q-block 2)
T_EYE = 0.010     # transpose identity (first fini)
T_WO = 0.014      # wo (first oproj units)

_CACHE = {}


def _build():
    import concourse.mybir as mybir
    import concourse.tile as tile
    from concourse import bacc

    f32 = mybir.dt.float32
    bf16 = mybir.dt.bfloat16
    f8 = mybir.dt.float8e4
    Exp = mybir.ActivationFunctionType.Exp
    DR = mybir.MatmulPerfMode.DoubleRow

    nc = bacc.Bacc("TRN2", target_bir_lowering=False, debug=False, num_devices=8)

    xh = nc.dram_tensor("xh", [D, S], f8, kind="ExternalInput").ap()
    xl = nc.dram_tensor("xl", [D, S], f8, kind="ExternalInput").ap()
    wqh = nc.dram_tensor("wqh", [D, QC], f8, kind="ExternalInput").ap()
    wql = nc.dram_tensor("wql", [D, QC], f8, kind="ExternalInput").ap()
    wkh = nc.dram_tensor("wkh", [D, KVC], f8, kind="ExternalInput").ap()
    wkl = nc.dram_tensor("wkl", [D, KVC], f8, kind="ExternalInput").ap()
    wvh = nc.dram_tensor("wvh", [D, KVC], f8, kind="ExternalInput").ap()
    wvl = nc.dram_tensor("wvl", [D, KVC], f8, kind="ExternalInput").ap()
    woh = nc.dram_tensor("woh", [QC, D], f8, kind="ExternalInput").ap()
    wol = nc.dram_tensor("wol", [QC, D], f8, kind="ExternalInput").ap()
    eye = nc.dram_tensor("eye", [128, 128], bf16, kind="ExternalInput").ap()
    out = nc.dram_tensor("out", [S, D], f32, kind="ExternalOutput").ap()

    MUL = mybir.AluOpType.mult
    SUB = mybir.AluOpType.subtract

    with tile.TileContext(nc) as tc:
        with tc.tile_pool(name="const", bufs=1) as const, \
             tc.tile_pool(name="mm", bufs=2, space="PSUM") as mm, \
             tc.tile_pool(name="qpp", bufs=1, space="PSUM") as qpp, \
             tc.tile_pool(name="op", bufs=1, space="PSUM") as opp, \
             tc.tile_pool(name="av", bufs=1, space="PSUM") as av, \
             tc.tile_pool(name="tr", bufs=1, space="PSUM") as trp, \
             tc.tile_pool(name="qs", bufs=3) as qs, \
             tc.tile_pool(name="pp", bufs=6) as pp, \
             tc.tile_pool(name="dn", bufs=8) as dn, \
             tc.tile_pool(name="ar", bufs=2) as ar, \
             tc.tile_pool(name="ev", bufs=6) as ev:

            # ---------------- resident inputs ----------------
            wkh_all = const.tile([128, DC, KVC], f8, tag="wkh_all")
            nc.sync.dma_start(out=wkh_all[:], in_=wkh.rearrange("(c p) n -> p c n", p=128))
            wkl_all = const.tile([128, DC, KVC], f8, tag="wkl_all")
            nc.sync.dma_start(out=wkl_all[:], in_=wkl.rearrange("(c p) n -> p c n", p=128))
            wqh_all = const.tile([128, DC, QC], f8, tag="wqh_all")
            wql_all = const.tile([128, DC, QC], f8, tag="wql_all")
            wqh_re = wqh.rearrange("(c p) n -> p c n", p=128)
            wql_re = wql.rearrange("(c p) n -> p c n", p=128)
            nc.sync.dma_start(out=wqh_all[:, :, 0:128], in_=wqh_re[:, :, 0:128])
            nc.sync.dma_start(out=wql_all[:, :, 0:128], in_=wql_re[:, :, 0:128])
            xh_all = const.tile([128, DC, S], f8, tag="xh_all")
            xl_all = const.tile([128, DC, S], f8, tag="xl_all")
            xh_re = xh.rearrange("(c p) s -> p c s", p=128)
            xl_re = xl.rearrange("(c p) s -> p c s", p=128)
            # x streams on the (otherwise idle) gpsimd DMA queue in seq-column
            # blocks (all DC rows per block): k-proj s4 / q-proj jq blocks
            # depend only on their own columns, so compute streams behind the
            # DMA with ~0.5MB granularity. Blocks 1-3 are emitted inside the
            # k-proj loop so the kdup copies (same queue) interleave behind
            # the x block they depend on without head-of-line blocking.
            def xdma(c):
                sl = slice(c * 512, (c + 1) * 512)
                nc.gpsimd.dma_start(out=xh_all[:, :, sl], in_=xh_re[:, :, sl])
                nc.gpsimd.dma_start(out=xl_all[:, :, sl], in_=xl_re[:, :, sl])

            xdma(0)
            wvh_all = const.tile([128, DC, KVC], f8, tag="wvh_all")
            wvl_all = const.tile([128, DC, KVC], f8, tag="wvl_all")
            nc.sync.dma_start(out=wvh_all[:],
                              in_=wvh.rearrange("(c p) n -> p c n", p=128))
            nc.sync.dma_start(out=wvl_all[:],
                              in_=wvl.rearrange("(c p) n -> p c n", p=128))
            eye_t = const.tile([128, 128], bf16, tag="eye_t")
            woh_all = const.tile([128, QC // 128, D], f8, tag="woh_all")
            wol_all = const.tile([128, QC // 128, D], f8, tag="wol_all")

            # persistent intermediates
            kdup = const.tile([128, 2, S], bf16, tag="kdup")  # kv head x both halves
            k_sb = const.tile([128, S], bf16, tag="k_sb")
            vv = const.tile([128, SC, 2, 80], bf16, tag="vv")  # [k, sc, kv, 64v+ones]
            at8h = const.tile([128, 4, S], f8, tag="at8h")    # attn out^T hi digit
            at8l = const.tile([128, 4, S], f8, tag="at8l")    # attn out^T lo digit
            bias_m2 = const.tile([128, 1], f32, tag="bias_m2")
            nc.vector.memset(bias_m2[:], -2.0)
            warm = const.tile([128, 1], bf16, tag="warm")
            nc.scalar.activation(warm[:], bias_m2[:], Exp, bias=bias_m2[:],
                                 scale=SCALE)
            nc.vector.memset(vv[:, :, :, 64:65], 1.0)

            # ---------------- k projection: k^T [128 kcols, S] ----------------
            for s4 in range(4):
                if s4 + 1 < 4:
                    xdma(s4 + 1)
                sl = slice(s4 * 512, (s4 + 1) * 512)
                kp = mm.tile([128, 2, 512], f32, tag="mm4k")
                first = True
                for wk_t, x_t in ((wkh_all, xh_all), (wkl_all, xh_all),
                                  (wkh_all, xl_all)):
                    for d in range(DC // 2):
                        nc.tensor.matmul(kp[:, 0, :],
                                         wk_t[:, 2 * d:2 * d + 2, :],
                                         x_t[:, 2 * d:2 * d + 2, sl],
                                         start=first,
                                         stop=(wk_t is wkh_all
                                               and x_t is xl_all
                                               and d == DC // 2 - 1),
                                         perf_mode=DR)
                        first = False
                nc.vector.tensor_scalar(k_sb[:, sl], kp[:, 0, :],
                                        1.0 / WS, None, MUL)
                # duplicate each kv head into both partition halves (per s4,
                # so block-0 scores can start before the later s4 finish)
                for kv in range(2):
                    for half in range(2):
                        nc.gpsimd.dma_start(
                            out=kdup[half * 64:half * 64 + 64, kv, sl],
                            in_=k_sb[kv * 64:kv * 64 + 64, sl])

            # ------------- q-proj pipeline stage -------------
            blocks = [(jq, qm) for jq in range(NJQ) for qm in range(4)]
            q_sb_of = {}

            def qstage(i):
                jq, qm = blocks[i]
                qsl = slice(jq * QB, (jq + 1) * QB)
                qp = qpp.tile([128, 512], f32, tag="qp")
                first = True
                for wq_t, x_t in ((wqh_all, xh_all), (wql_all, xh_all),
                                  (wqh_all, xl_all)):
                    for d in range(DC // 2):
                        nc.tensor.matmul(qp[:],
                                         wq_t[:, 2 * d:2 * d + 2,
                                              qm * 128:(qm + 1) * 128],
                                         x_t[:, 2 * d:2 * d + 2, qsl],
                                         start=first,
                                         stop=(wq_t is wqh_all
                                               and x_t is xl_all
                                               and d == DC // 2 - 1),
                                         perf_mode=DR)
                        first = False
                q_sb = qs.tile([128, QB], bf16, tag="q_sb")
                nc.vector.tensor_scalar(q_sb[:], qp[:], 1.0 / WS, None, MUL)
                q_sb_of[i] = q_sb

            qstage(0)
            # deferred loads ride the ACT DMA queue (idle until exps start,
            # and input loads have no waits so they can't head-of-line block
            # it); the SP queue stays clear for the k_sb-gated kdup copies.
            with tc.tile_wait_until(T_WQ):
                nc.sync.dma_start(out=wqh_all[:, :, 128:QC],
                                    in_=wqh_re[:, :, 128:QC])
                nc.sync.dma_start(out=wql_all[:, :, 128:QC],
                                    in_=wql_re[:, :, 128:QC])
            with tc.tile_wait_until(T_EYE):
                nc.sync.dma_start(out=eye_t[:], in_=eye)
            with tc.tile_wait_until(T_WO):
                nc.sync.dma_start(out=woh_all[:],
                                    in_=woh.rearrange("(c p) n -> p c n", p=128))
                nc.sync.dma_start(out=wol_all[:],
                                    in_=wol.rearrange("(c p) n -> p c n", p=128))

            # ---------------- v projection (deferred fillers) ----------------
            def vproj_unit(sc):
                vp = opp.tile([128, 512], f32, tag="op", name="vp")
                first = True
                for x_t, wv_t in ((xh_all, wvh_all), (xh_all, wvl_all),
                                  (xl_all, wvh_all)):
                    for d in range(DC // 2):
                        nc.tensor.matmul(vp[:, 0:128],
                                         x_t[:, 2 * d:2 * d + 2,
                                             sc * 128:(sc + 1) * 128],
                                         wv_t[:, 2 * d:2 * d + 2, :],
                                         start=first,
                                         stop=(x_t is xl_all
                                               and d == DC // 2 - 1),
                                         perf_mode=DR)
                        first = False
                nc.vector.tensor_scalar(vv[:, sc, 0, 0:64], vp[:, 0:64],
                                        1.0 / WS, None, MUL)
                nc.vector.tensor_scalar(vv[:, sc, 1, 0:64], vp[:, 64:128],
                                        1.0 / WS, None, MUL)

            vfill = list(range(SC))

            # ------------- fused attention + o_proj -------------
            def oproj_unit(sm, q4, tail=0):
                if tail % 3 == 1:
                    op2 = mm.tile([128, 2, 512], f32, tag="mm4k", name="opt")
                    op = op2[:, 0, :]
                elif tail % 3 == 2:
                    op2 = mm.tile([128, 2, 512], f32, tag="mm4k", name="opt2")
                    op = op2[:, 1, :]
                else:
                    op = opp.tile([128, 512], f32, tag="op", name="op")
                o = q4 * 512
                first = True
                for a_t, wo_t in ((at8h, woh_all), (at8h, wol_all),
                                  (at8l, woh_all)):
                    for c in range(2):
                        nc.tensor.matmul(op[:],
                                         a_t[:, 2 * c:2 * c + 2,
                                             sm * 128:(sm + 1) * 128],
                                         wo_t[:, 2 * c:2 * c + 2, o:o + 512],
                                         start=first,
                                         stop=(a_t is at8l and c == 1),
                                         perf_mode=DR)
                        first = False
                o_sb = ev.tile([128, 512], f32, tag="o_sb")
                nc.vector.tensor_scalar(o_sb[:], op[:], 1.0 / (WS * AS),
                                        None, MUL)
                nc.sync.dma_start(
                    out=out[sm * 128:(sm + 1) * 128, q4 * 512:(q4 + 1) * 512],
                    in_=o_sb[:])

            # Flat task stream over (block, head, kc-pair); attn@v trails by
            # ATTNV_LAG tasks so its exp-wait never blocks scores in the PE
            # FIFO; o_proj/v_proj units fill PE slack mid-head.
            ATTNV_LAG = 4
            pending = []
            attnv_q = []
            fini_q = []

            def drain_attnv():
                o_ps, p_ap, pr, kv, fini = attnv_q.pop(0)
                for j2 in range(2):
                    kc = 2 * pr + j2
                    for qc in range(4):
                        nc.tensor.matmul(
                            o_ps[:, qc, 0:65],
                            p_ap[:, j2, qc * 128:(qc + 1) * 128],
                            vv[:, kc, kv, 0:65],
                            start=(kc == 0 and qc == 0),
                            stop=(kc == SC - 1 and qc == 3),
                            skip_group_check=True)
                if fini is not None:
                    fini_q.append(fini)

            def drain_fini():
                o_ps, hb, cc, jq = fini_q.pop(0)
                atr = ar.tile([128, 4, HD], bf16, tag="atr")
                for qc in range(4):
                    rc = dn.tile([128, 1], f32, tag="rc")
                    nc.vector.reciprocal(rc[:], o_ps[:, qc, 64:65])
                    # atr = (o_ps / denom) * AS  (AS centers the fp8 split)
                    nc.vector.tensor_scalar(atr[:, qc, :], o_ps[:, qc, 0:64],
                                            rc[:], AS, MUL, MUL)
                tr = trp.tile([128, 4, 128], bf16, tag="tr")
                for qc in range(4):
                    nc.tensor.matmul(tr[hb:hb + 64, qc, :], atr[:, qc, :],
                                     eye_t[:], is_transpose=True,
                                     tile_position=(0, hb))
                sl = slice(jq * QB, (jq + 1) * QB)
                nc.vector.tensor_copy(at8h[hb:hb + 64, cc, sl],
                                      tr[hb:hb + 64, :, :])
                nc.vector.tensor_tensor(at8l[hb:hb + 64, cc, sl],
                                        tr[hb:hb + 64, :, :],
                                        at8h[hb:hb + 64, cc, sl], SUB)

            for i, (jq, qm) in enumerate(blocks):
                if i + 1 < len(blocks):
                    qstage(i + 1)
                q_sb = q_sb_of.pop(i)
                for h2 in range(2):
                    l = 2 * qm + h2
                    kv = l // 4
                    hb = 64 * (l % 2)
                    cc = l // 2
                    qb = 64 * h2
                    o_ps = av.tile([128, 4, 80], f32, tag="o_ps")
                    for pr in range(8):
                        scp = mm.tile([128, 2, 512], f32, tag="mm4k")
                        for j2 in range(2):
                            kc = 2 * pr + j2
                            nc.tensor.matmul(
                                scp[:, j2, :],
                                kdup[qb:qb + 64, kv, kc * 128:(kc + 1) * 128],
                                q_sb[qb:qb + 64, :],
                                start=True, stop=True)
                        p4 = pp.tile([128, 2, QB], bf16, tag="p4")
                        nc.scalar.activation(p4[:], scp[:], Exp,
                                             bias=bias_m2[:], scale=SCALE)
                        fini = (o_ps, hb, cc, jq) if pr == 7 else None
                        attnv_q.append((o_ps, p4, pr, kv, fini))
                        if len(attnv_q) > ATTNV_LAG:
                            drain_attnv()
                        while fini_q:
                            drain_fini()
                        for _ in range(2):
                            if vfill:
                                vproj_unit(vfill.pop(0))
                        if pr in (3, 6) and pending:
                            oproj_unit(*pending.pop(0))
                if qm == 3:
                    pending.extend(((jq * 4 + smq, q4)
                                    for smq in range(4) for q4 in range(4)))
            while attnv_q:
                drain_attnv()
            while fini_q:
                drain_fini()
            for n, (sm, q4) in enumerate(pending):
                oproj_unit(sm, q4, tail=1 + (n % 3))

    nc.compile()
    return nc


def _get_nc():
    if "nc" not in _CACHE:
        _CACHE["nc"] = _build()
    return _CACHE["nc"]


def _split8(a):
    e4m3 = ml_dtypes.float8_e4m3
    hi = a.astype(e4m3)
    lo = (a - hi.astype(np.float32)).astype(e4m3)
    return hi, lo


def kernel(x, wq, wk, wv, wo):
    from concourse.bass_utils import run_bass_kernel_spmd

    bf16 = ml_dtypes.bfloat16
    nc = _get_nc()

    xnp, wqnp, wknp, wvnp, wonp = (
        np.asarray(a, dtype=np.float32) for a in (x, wq, wk, wv, wo))
    eye = np.eye(128, dtype=bf16)

    xs = [_split8(np.ascontiguousarray(xnp[b].T)) for b in range(2)]
    gsplits = []
    for g in range(4):
        gsplits.append({
            "wq": _split8(wqnp[:, g * QC:(g + 1) * QC] * WS),
            "wk": _split8(wknp[:, g * KVC:(g + 1) * KVC] * WS),
            "wv": _split8(wvnp[:, g * KVC:(g + 1) * KVC] * WS),
            "wo": _split8(wonp[g * QC:(g + 1) * QC, :] * WS),
        })

    in_maps = []
    for core in range(8):
        b, g = core // 4, core % 4
        gs = gsplits[g]
        in_maps.append({
            "xh": xs[b][0], "xl": xs[b][1],
            "wqh": gs["wq"][0], "wql": gs["wq"][1],
            "wkh": gs["wk"][0], "wkl": gs["wk"][1],
            "wvh": gs["wv"][0], "wvl": gs["wv"][1],
            "woh": gs["wo"][0], "wol": gs["wo"][1],
            "eye": eye,
        })

    res = run_bass_kernel_spmd(nc, in_maps, core_ids=list(range(8)))
    outs = [res.results[c]["out"] for c in range(8)]
    full = np.empty((2, S, D), np.float32)
    full[0] = outs[0] + outs[1] + outs[2] + outs[3]
    full[1] = outs[4] + outs[5] + outs[6] + outs[7]
    return full
